# revision 6
# baseline (speedup 1.0000x reference)
"""nn_GCNConv Trainium2 Bass kernel (8 NeuronCores, SPMD, no collectives).

Computation: out = segment_sum(features[src], dst, N) @ W + b
  features [10000,128] f32, edge_index [2,640000] i64, W [128,256], b [256]

Strategy (dense-adjacency SpMM, dst-node sharding -> no cross-core reduce):
  segment_sum(features[src], dst) == A^T @ features, where A[s,d] is the
  number of edges s->d (small integer counts).  Instead of gathering 80k
  random 256B feature rows per core (SWDGE descriptor-rate bound, ~300us),
  the host builds the dense count matrix A once from edge_index (integer
  work only) and each core STREAMS its dst-slab of A sequentially at full
  HBM bandwidth:

  - dst axis padded to 10240 = 8 cores x 1280 columns; core c owns
    dst [1280c, 1280c+1280).
  - src axis padded to 10112 = 79 chunks of 128 rows.
  - A slab per core, split into three column groups (512/512/256 dst) so
    each group's output projection overlaps the next group's accumulation.
    fp8e4 (e4m3) represents the small integer counts exactly -> no
    quantization error on A.
  - Features are replicated to every core as [128 part, 79 chunk, 128 feat]
    bf16 (host-rounded; load interleaved with the first A pass).
  - PE per group: aggT[feat,dstg] += featbf[:,k,:]^T @ A_g[:,k,:]
    accumulated over the 79 chunks into one PSUM bank (f32).
  - Projection per 128-dst window (pipelined PSUM->SBUF copies):
    out = aggT^T @ W (bf16) + b (DVE add), DMA'd out per window.
  - Host concatenates the 8 per-core [1280,256] outputs, truncates to
    10000 rows.  Only integer counting/packing happens on the host; all
    float arithmetic on feature values runs on device.
"""

import sys

import numpy as np

_TRN_REPO = "/opt/trn_rl_repo"
if _TRN_REPO not in sys.path:
    sys.path.insert(0, _TRN_REPO)

import ml_dtypes  # noqa: E402

import concourse.bass as bass  # noqa: E402
import concourse.mybir as mybir  # noqa: E402
import concourse.tile as tile  # noqa: E402
from concourse import bacc, bass_utils  # noqa: E402

# ---------------------------------------------------------------------------
# Workaround: this walrus build rejects >1 sync-wait on a CTRL instruction
# ("Too many sync wait commands"). Tile's tail drain attaches a wait for every
# live sem lane to one InstDrain; chunk them across single-wait nops instead.
import re as _re  # noqa: E402

import bass_rust as _bass_rust  # noqa: E402


def _clock_ticks(vc):
    m = _re.search(r"\[([0-9, ]*)\]", repr(vc))
    return [int(x) for x in m.group(1).split(",")] if m.group(1).strip() else []


def _drain_and_barrier(self, tick_clock, wait_clock):
    ticks = _clock_ticks(tick_clock.global_clock)
    nz = [(i, t) for i, t in enumerate(ticks) if t > 0]
    for i, t in nz:
        vc = _bass_rust.VectorClock()
        vc.require_at_least(i, t)
        nop = self.nc.sync.nop(nofuse=True, hint="tail_wait")
        wait_clock.add_sem_waits(nop.ins, tile.ScopedClock({None: vc}))
    self.nc.sync.drain()  # waits already carried by the nops (SP FIFO order)
    self.nc.all_engine_barrier()
    assert self.sems is not None
    popped = self.nc._tile_sem_poison_stack.pop()
    assert popped is self._sem_poison
    self.nc.clear_and_free_semaphores(list(self.sems.allocated().values()))
    self.nc.all_engine_barrier()


tile.TileContext._drain_and_barrier = _drain_and_barrier
# ---------------------------------------------------------------------------

P = 128
C_IN = 128
C_OUT = 256
N_NODES = 10000
N_CORES = 8
WPC = 10                      # dst windows (of 128 nodes) per core
NCH = 79                      # src chunks of 128 (10112 >= 10000)
N_SRC_PAD = NCH * P           # 10112
DST_PC = WPC * P              # 1280 dst columns per core
CG_W = [512, 512, 256]        # column-group widths (<= 1 PSUM bank each)
CG_O = [0, 512, 1024]         # column-group dst offsets
ACH = 16                      # A chunks per DMA
FGROUPS = [(0, 4), (4, 16), (16, 40), (40, 79)]  # feat (bf16) load groups


def _build_kernel():
    nc = bacc.Bacc("TRN2")
    dt = mybir.dt

    featf_d = nc.dram_tensor("featf", [P, NCH, C_IN], dt.bfloat16, kind="ExternalInput")
    ag_d = [
        nc.dram_tensor(f"a{g}", [P, NCH, CG_W[g]], dt.float8e4, kind="ExternalInput")
        for g in range(3)
    ]
    w_d = nc.dram_tensor("w", [C_IN, C_OUT], dt.float32, kind="ExternalInput")
    bb_d = nc.dram_tensor("bb", [P, C_OUT], dt.float32, kind="ExternalInput")
    out_d = nc.dram_tensor("out", [DST_PC, C_OUT], dt.float32, kind="ExternalOutput")

    with tile.TileContext(nc) as tc:
        with (
            tc.tile_pool(name="consts", bufs=1) as cpool,
            tc.tile_pool(name="feat", bufs=1) as fpool,
            tc.tile_pool(name="astream", bufs=3) as apool,
            tc.tile_pool(name="aggs", bufs=2) as spool,
            tc.tile_pool(name="outs", bufs=3) as opool,
            tc.tile_pool(name="psagg", bufs=2, space="PSUM") as psa,
            tc.tile_pool(name="psout", bufs=2, space="PSUM") as pso,
        ):
            w32 = cpool.tile([C_IN, C_OUT], dt.float32)
            wbf = cpool.tile([C_IN, C_OUT], dt.bfloat16)
            bb_s = cpool.tile([P, C_OUT], dt.float32)

            fbf = fpool.tile([P, NCH, C_IN], dt.bfloat16)
            agroups = [(a, min(a + ACH, NCH)) for a in range(0, NCH, ACH)]

            def load_feat_group(i):
                a0, a1 = FGROUPS[i]
                nc.sync.dma_start(out=fbf[:, a0:a1, :], in_=featf_d[:, a0:a1, :])

            # head: first small feat group so the first matmul starts ASAP,
            # then interleave remaining feat loads with the first A stream
            load_feat_group(0)
            nc.sync.dma_start(out=w32[:], in_=w_d[:])
            nc.sync.dma_start(out=bb_s[:], in_=bb_d[:])
            nc.vector.tensor_copy(wbf[:], w32[:])

            for g in range(3):
                wg = CG_W[g]
                agg_p = psa.tile([P, 512], dt.float32, tag="agg")  # 1 bank
                for gi, (a0, a1) in enumerate(agroups):
                    if g == 0 and gi + 1 < len(FGROUPS):
                        load_feat_group(gi + 1)
                    at = apool.tile([P, ACH, 512], dt.float8e4, tag="a")
                    nc.sync.dma_start(
                        out=at[:, : a1 - a0, :wg], in_=ag_d[g][:, a0:a1, :]
                    )
                    for k in range(a0, a1):
                        nc.tensor.matmul(
                            agg_p[:, :wg],
                            lhsT=fbf[:, k, :],
                            rhs=at[:, k - a0, :wg],
                            start=(k == 0),
                            stop=(k == NCH - 1),
                        )
                # project this group's windows (pipelined 128-col copies)
                for wi in range(wg // P):
                    w = CG_O[g] // P + wi
                    aggs = spool.tile([P, P], dt.bfloat16, tag="aggs", bufs=4)
                    nc.scalar.copy(aggs[:], agg_p[:, wi * P : (wi + 1) * P])
                    out_p = pso.tile([P, C_OUT], dt.float32, tag="op")
                    nc.tensor.matmul(
                        out_p[:], lhsT=aggs[:], rhs=wbf[:], start=True, stop=True
                    )
                    out_t = opool.tile([P, C_OUT], dt.float32, tag="ot")
                    nc.vector.tensor_add(out_t[:], out_p[:], bb_s[:])
                    nc.sync.dma_start(out=out_d[w * P : (w + 1) * P, :], in_=out_t[:])

    nc.compile()
    return nc


def _prep_inputs(features, edge_index, W, b):
    """Host-side packing: dense count matrix A from edge_index (integer
    work only), per-core dst slabs split into column groups, replicated
    features/weights."""
    src = np.asarray(edge_index[0]).astype(np.int64)
    dst = np.asarray(edge_index[1]).astype(np.int64)

    dst_pad = N_CORES * DST_PC  # 10240
    counts = np.zeros(N_SRC_PAD * dst_pad, dtype=np.uint8)
    np.add.at(counts, src * dst_pad + dst, 1)
    assert counts.max() <= 15  # e4m3 is exact for small ints
    # uint8 -> fp8e4 bytes via lookup (fast, exact)
    lut = np.arange(256, dtype=np.uint8).astype(ml_dtypes.float8_e4m3).view(np.uint8)
    # [src, dst] -> [part, chunk, dst]
    a_view = counts.reshape(NCH, P, dst_pad).transpose(1, 0, 2)

    feat_np = np.zeros((N_SRC_PAD, C_IN), dtype=ml_dtypes.bfloat16)
    feat_np[:N_NODES] = np.asarray(features, dtype=np.float32).astype(ml_dtypes.bfloat16)
    feat_np = np.ascontiguousarray(feat_np.reshape(NCH, P, C_IN).transpose(1, 0, 2))

    w_np = np.ascontiguousarray(np.asarray(W, dtype=np.float32))
    bb_np = np.tile(np.asarray(b, dtype=np.float32)[None, :], (P, 1))

    in_maps = []
    for c in range(N_CORES):
        m = {"featf": feat_np, "w": w_np, "bb": bb_np}
        base = c * DST_PC
        for g in range(3):
            a_c = np.ascontiguousarray(
                a_view[:, :, base + CG_O[g] : base + CG_O[g] + CG_W[g]]
            )
            m[f"a{g}"] = lut[a_c].view(ml_dtypes.float8_e4m3)
        in_maps.append(m)
    return in_maps


_KERNEL_CACHE: dict = {}


def _get_kernel():
    if "nc" not in _KERNEL_CACHE:
        _KERNEL_CACHE["nc"] = _build_kernel()
    return _KERNEL_CACHE["nc"]


def kernel(features, edge_index, W, b):
    features = np.asarray(features, dtype=np.float32)
    edge_index = np.asarray(edge_index)
    W = np.asarray(W, dtype=np.float32)
    b = np.asarray(b, dtype=np.float32)
    assert features.shape == (N_NODES, C_IN), features.shape
    assert W.shape == (C_IN, C_OUT) and b.shape == (C_OUT,)

    in_maps = _prep_inputs(features, edge_index, W, b)
    nc = _get_kernel()
    res = bass_utils.run_bass_kernel_spmd(nc, in_maps, core_ids=list(range(N_CORES)))
    out = np.concatenate([res.results[c]["out"] for c in range(N_CORES)], axis=0)
    return np.ascontiguousarray(out[:N_NODES]).astype(np.float32)


# revision 7
# speedup vs baseline: 1.0741x; 1.0741x over previous
"""nn_GCNConv Trainium2 Bass kernel (8 NeuronCores, SPMD, no collectives).

Computation: out = segment_sum(features[src], dst, N) @ W + b
  features [10000,128] f32, edge_index [2,640000] i64, W [128,256], b [256]

Strategy (dense-adjacency SpMM, dst-node sharding -> no cross-core reduce):
  segment_sum(features[src], dst) == A^T @ features, where A[s,d] is the
  number of edges s->d (small integer counts).  Instead of gathering 80k
  random 256B feature rows per core (SWDGE descriptor-rate bound, ~300us),
  the host builds the dense count matrix A once from edge_index (integer
  work only) and each core STREAMS its dst-slab of A sequentially at full
  HBM bandwidth:

  - dst axis padded to 10240 = 8 cores x 1280 columns; core c owns
    dst [1280c, 1280c+1280).
  - src axis padded to 10240 = 80 chunks of 128 rows (even, for DoubleRow
    chunk pairing).
  - A slab per core in fp8e4 (e4m3 represents the integer counts exactly,
    no quantization error), split into three column groups (512/512/256
    dst = one PSUM bank) so each group's output projection overlaps the
    next group's accumulation.
  - Features are replicated to every core as a two-component fp8 split
    feat = hi + lo (hi = fp8(feat), lo = fp8(feat - hi)), interleaved
    [128 part, 80 chunk, 2, 128 feat].  The sum carries ~bf16 accuracy
    (measured 2.6e-3 rel vs 3.0e-3 for bf16) while every matmul operand is
    fp8 -> MatmulPerfMode.DoubleRow runs the PE at 2x (157 TF/s).
  - PE per column group: for each chunk pair (k,k+1), two DoubleRow
    matmuls accumulate hi_k^T A_k + hi_{k+1}^T A_{k+1} and the lo pair
    into PSUM f32: aggT[feat,dstg] = (hi+lo)^T A = feat^T A.
  - Projection per 128-dst window (pipelined PSUM->SBUF copies):
    out = aggT^T @ W (bf16) + b (DVE add), DMA'd out per window.
  - DMA-queue split: the SP HWDGE queue carries only the A stream; the
    Activation HWDGE queue carries feature/const loads and output writes,
    so the two DGE pipelines overlap (single-queue entry latency ~2.5us
    was the dominant stall in earlier versions).
  - Host concatenates the 8 per-core [1280,256] outputs, truncates to
    10000 rows.  Only integer counting/packing and the fp8 hi/lo encoding
    happen on the host; all arithmetic on feature values runs on device.
"""

import sys

import numpy as np

_TRN_REPO = "/opt/trn_rl_repo"
if _TRN_REPO not in sys.path:
    sys.path.insert(0, _TRN_REPO)

import ml_dtypes  # noqa: E402

import concourse.bass as bass  # noqa: E402
import concourse.mybir as mybir  # noqa: E402
import concourse.tile as tile  # noqa: E402
from concourse import bacc, bass_utils  # noqa: E402

# ---------------------------------------------------------------------------
# Workaround: this walrus build rejects >1 sync-wait on a CTRL instruction
# ("Too many sync wait commands"). Tile's tail drain attaches a wait for every
# live sem lane to one InstDrain; chunk them across single-wait nops instead.
import re as _re  # noqa: E402

import bass_rust as _bass_rust  # noqa: E402


def _clock_ticks(vc):
    m = _re.search(r"\[([0-9, ]*)\]", repr(vc))
    return [int(x) for x in m.group(1).split(",")] if m.group(1).strip() else []


def _drain_and_barrier(self, tick_clock, wait_clock):
    ticks = _clock_ticks(tick_clock.global_clock)
    nz = [(i, t) for i, t in enumerate(ticks) if t > 0]
    for i, t in nz:
        vc = _bass_rust.VectorClock()
        vc.require_at_least(i, t)
        nop = self.nc.sync.nop(nofuse=True, hint="tail_wait")
        wait_clock.add_sem_waits(nop.ins, tile.ScopedClock({None: vc}))
    self.nc.sync.drain()  # waits already carried by the nops (SP FIFO order)
    self.nc.all_engine_barrier()
    assert self.sems is not None
    popped = self.nc._tile_sem_poison_stack.pop()
    assert popped is self._sem_poison
    self.nc.clear_and_free_semaphores(list(self.sems.allocated().values()))
    self.nc.all_engine_barrier()


tile.TileContext._drain_and_barrier = _drain_and_barrier
# ---------------------------------------------------------------------------

P = 128
C_IN = 128
C_OUT = 256
N_NODES = 10000
N_CORES = 8
WPC = 10                      # dst windows (of 128 nodes) per core
NCH = 80                      # src chunks of 128 (10240 >= 10000, even)
N_SRC_PAD = NCH * P           # 10240
DST_PC = WPC * P              # 1280 dst columns per core
CG_W = [512, 512, 256]        # column-group widths (<= 1 PSUM bank each)
CG_O = [0, 512, 1024]         # column-group dst offsets
ACH = 16                      # A chunks per DMA (even, for DR pairing)
FGROUPS = [(0, 16), (16, 48), (48, 80)]  # feature (hi/lo fp8) load groups


def _build_kernel():
    nc = bacc.Bacc("TRN2")
    dt = mybir.dt
    DR = mybir.MatmulPerfMode.DoubleRow

    f8_d = nc.dram_tensor("f8", [P, NCH, 2, C_IN], dt.float8e4, kind="ExternalInput")
    ag_d = [
        nc.dram_tensor(f"a{g}", [P, NCH, CG_W[g]], dt.float8e4, kind="ExternalInput")
        for g in range(3)
    ]
    # merged consts: [:, :256] = W (f32), [:, 256:] = bias rows (f32)
    wb_d = nc.dram_tensor("wb", [P, 2 * C_OUT], dt.float32, kind="ExternalInput")
    out_d = nc.dram_tensor("out", [DST_PC, C_OUT], dt.float32, kind="ExternalOutput")

    with tile.TileContext(nc) as tc:
        with (
            tc.tile_pool(name="consts", bufs=1) as cpool,
            tc.tile_pool(name="feat", bufs=1) as fpool,
            tc.tile_pool(name="astream", bufs=3) as apool,
            tc.tile_pool(name="aggs", bufs=2) as spool,
            tc.tile_pool(name="outs", bufs=3) as opool,
            tc.tile_pool(name="psagg", bufs=2, space="PSUM") as psa,
            tc.tile_pool(name="psout", bufs=2, space="PSUM") as pso,
        ):
            f8 = fpool.tile([P, NCH, 2, C_IN], dt.float8e4)
            wb_s = cpool.tile([P, 2 * C_OUT], dt.float32)
            wbf = cpool.tile([C_IN, C_OUT], dt.bfloat16)
            agroups = [(a, min(a + ACH, NCH)) for a in range(0, NCH, ACH)]

            def load_feat_group(i):
                a0, a1 = FGROUPS[i]
                nc.scalar.dma_start(out=f8[:, a0:a1, :, :], in_=f8_d[:, a0:a1, :, :])

            # Activation HWDGE queue: features, consts, (later) outputs.
            # SP HWDGE queue: only the A stream.
            load_feat_group(0)
            nc.scalar.dma_start(out=wb_s[:], in_=wb_d[:])
            nc.vector.tensor_copy(wbf[:], wb_s[:, :C_OUT])
            load_feat_group(1)
            load_feat_group(2)

            for g in range(3):
                wg = CG_W[g]
                agg_p = psa.tile([P, 512], dt.float32, tag="agg")  # 1 bank
                for gi, (a0, a1) in enumerate(agroups):
                    at = apool.tile([P, ACH, 512], dt.float8e4, tag="a")
                    nc.sync.dma_start(
                        out=at[:, : a1 - a0, :wg], in_=ag_d[g][:, a0:a1, :]
                    )
                    for k in range(a0, a1, 2):
                        j = k - a0
                        for h in range(2):  # hi then lo component
                            nc.tensor.matmul(
                                agg_p[:, :wg],
                                lhsT=f8[:, k : k + 2, h, :],
                                rhs=at[:, j : j + 2, :wg],
                                start=(k == 0 and h == 0),
                                stop=(k == NCH - 2 and h == 1),
                                perf_mode=DR,
                            )
                # project this group's windows (pipelined 128-col copies)
                for wi in range(wg // P):
                    w = CG_O[g] // P + wi
                    aggs = spool.tile([P, P], dt.bfloat16, tag="aggs", bufs=4)
                    nc.scalar.copy(aggs[:], agg_p[:, wi * P : (wi + 1) * P])
                    out_p = pso.tile([P, C_OUT], dt.float32, tag="op")
                    nc.tensor.matmul(
                        out_p[:], lhsT=aggs[:], rhs=wbf[:], start=True, stop=True
                    )
                    out_t = opool.tile([P, C_OUT], dt.float32, tag="ot")
                    nc.vector.tensor_add(out_t[:], out_p[:], wb_s[:, C_OUT:])
                    nc.scalar.dma_start(
                        out=out_d[w * P : (w + 1) * P, :], in_=out_t[:]
                    )

    nc.compile()
    return nc


def _prep_inputs(features, edge_index, W, b):
    """Host-side packing: dense count matrix A from edge_index (integer
    work only), per-core dst slabs split into column groups, fp8 hi/lo
    feature encoding, replicated weights."""
    src = np.asarray(edge_index[0]).astype(np.int64)
    dst = np.asarray(edge_index[1]).astype(np.int64)
    fp8 = ml_dtypes.float8_e4m3

    dst_pad = N_CORES * DST_PC  # 10240
    counts = np.zeros(N_SRC_PAD * dst_pad, dtype=np.uint8)
    np.add.at(counts, src * dst_pad + dst, 1)
    assert counts.max() <= 15  # e4m3 is exact for small ints
    # uint8 -> fp8e4 bytes via lookup (fast, exact)
    lut = np.arange(256, dtype=np.uint8).astype(fp8).view(np.uint8)
    # [src, dst] -> [part, chunk, dst]
    a_view = counts.reshape(NCH, P, dst_pad).transpose(1, 0, 2)

    feat32 = np.zeros((N_SRC_PAD, C_IN), dtype=np.float32)
    feat32[:N_NODES] = np.asarray(features, dtype=np.float32)
    hi = feat32.astype(fp8)
    lo = (feat32 - hi.astype(np.float32)).astype(fp8)
    f8_np = np.ascontiguousarray(
        np.stack([hi, lo], axis=1)  # [src, 2, C_IN]
        .reshape(NCH, P, 2, C_IN)
        .transpose(1, 0, 2, 3)
    )

    wb_np = np.empty((P, 2 * C_OUT), dtype=np.float32)
    wb_np[:, :C_OUT] = np.asarray(W, dtype=np.float32)
    wb_np[:, C_OUT:] = np.asarray(b, dtype=np.float32)[None, :]

    in_maps = []
    for c in range(N_CORES):
        m = {"f8": f8_np, "wb": wb_np}
        base = c * DST_PC
        for g in range(3):
            a_c = np.ascontiguousarray(
                a_view[:, :, base + CG_O[g] : base + CG_O[g] + CG_W[g]]
            )
            m[f"a{g}"] = lut[a_c].view(fp8)
        in_maps.append(m)
    return in_maps


_KERNEL_CACHE: dict = {}


def _get_kernel():
    if "nc" not in _KERNEL_CACHE:
        _KERNEL_CACHE["nc"] = _build_kernel()
    return _KERNEL_CACHE["nc"]


def kernel(features, edge_index, W, b):
    features = np.asarray(features, dtype=np.float32)
    edge_index = np.asarray(edge_index)
    W = np.asarray(W, dtype=np.float32)
    b = np.asarray(b, dtype=np.float32)
    assert features.shape == (N_NODES, C_IN), features.shape
    assert W.shape == (C_IN, C_OUT) and b.shape == (C_OUT,)

    in_maps = _prep_inputs(features, edge_index, W, b)
    nc = _get_kernel()
    res = bass_utils.run_bass_kernel_spmd(nc, in_maps, core_ids=list(range(N_CORES)))
    out = np.concatenate([res.results[c]["out"] for c in range(N_CORES)], axis=0)
    return np.ascontiguousarray(out[:N_NODES]).astype(np.float32)
